# revision 19
# baseline (speedup 1.0000x reference)
"""Bass/Trainium2 kernel for nn_BoundaryDecoderAttention.

Math (per batch row b):
  Fk    = tanh(H_r[b] @ V_w.T + V_b + h_tm1[b] @ Wa_w.T + Wa_b)   [T, D]
  s     = Fk @ v + c                                              [T]
  x     = clip(s, -15, 15) * m
  e     = exp(x) * m            (max-subtraction skipped: x <= 15 so exp is
                                 safe in fp32, and the 1e-6 epsilon term is
                                 ~1e-10 relative -- see analysis in test)
  beta  = e / (sum(e) + 1e-6)
  z     = beta @ H_r[b]                                           [D]

Sharding: data-parallel over batch, 8 rows per core, params replicated.
Device pipeline is fully "transposed": mm1 consumes host-pretransposed
H^T tiles (rhs), tanh runs on ACT with a per-partition bias, mm2 uses the
tanh tile as the *stationary* operand so scores are born spread across
128 partitions, and the z reduction streams natural-layout H blocks.
"""

import os
import sys
import time

for _p in ("/opt/trn_rl_repo", "/root/.axon_site/_ro/trn_rl_repo"):
    if os.path.isdir(_p) and _p not in sys.path:
        sys.path.insert(0, _p)

import numpy as np

import concourse.bass as bass
import concourse.bass_isa as bass_isa
from concourse import mybir
from concourse.tile import TileContext, ScopedClock

B, T, D = 64, 8192, 128
NCORES = 8
BL = B // NCORES          # 8 batch rows per core
NBLK = T // 128           # 64 t-blocks per row
NPAIR = 8                 # tanh super-chunks of 1024 cols
F32 = mybir.dt.float32

# compute dtype for the H-heavy matmuls ("float32" or "bfloat16")
MM_DTYPE = os.environ.get("BDA_MM_DTYPE", "bfloat16")
ABLATE = os.environ.get("BDA_ABLATE", "")

TIMING = {"exec_ns": None, "runs": []}


class PatchedTileContext(TileContext):
    """TileContext whose emitted instructions carry at most one sem wait.

    The walrus build in this container rejects instructions with more than
    one sync-wait command ("Too many sync wait commands"); excess waits are
    peeled onto dedicated same-engine NOPs placed just before the owner.
    """

    MAX_WAITS = 1

    def _lower_ordered_insts(self, ordered):
        for bbname in list(ordered.keys()):
            insts = ordered[bbname]
            new = []
            for inst in insts:
                si = inst.sync_info
                if si is not None and len(si.on_wait) > 1:
                    waits = list(si.on_wait)
                    for w in waits[:-1]:
                        nop = mybir.InstNoOp(
                            name=self.nc.get_next_instruction_name(),
                            sync_info=mybir.SyncInfo(on_wait=[w], on_update=[]),
                            bass_nofuse=True,
                            engine=inst.engine,
                        )
                        new.append(nop)
                    inst.sync_info = mybir.SyncInfo(
                        on_wait=[waits[-1]], on_update=list(si.on_update)
                    )
                new.append(inst)
            ordered[bbname] = new
        super()._lower_ordered_insts(ordered)

    def _drain_and_barrier(self, tick_clock, wait_clock):
        d0 = self.nc.sync.drain()
        wait_clock.add_sem_waits(d0.ins, ScopedClock({None: tick_clock.global_clock}))
        si0 = d0.ins.sync_info
        waits = list(si0.on_wait) if si0 is not None else []
        if len(waits) > self.MAX_WAITS:
            d0.ins.sync_info = mybir.SyncInfo(
                on_wait=waits[: self.MAX_WAITS], on_update=list(si0.on_update)
            )
            rest = waits[self.MAX_WAITS:]
            for i in range(0, len(rest), self.MAX_WAITS):
                d = self.nc.sync.drain()
                d.ins.sync_info = mybir.SyncInfo(
                    on_wait=rest[i : i + self.MAX_WAITS], on_update=[]
                )
        self.nc.all_engine_barrier()
        popped = self.nc._tile_sem_poison_stack.pop()
        assert popped is self._sem_poison
        self.nc.clear_and_free_semaphores(list(self.sems.allocated().values()))
        self.nc.all_engine_barrier()


def build_kernel(dtm, reps=1):
    nc = bass.Bass("TRN2")

    ht_t = nc.dram_tensor("ht_in", [BL, 128, T], dtm, kind="ExternalInput")
    hn_t = nc.dram_tensor("hn_in", [BL, 128, NBLK, D], dtm, kind="ExternalInput")
    mask_t = nc.dram_tensor("mask_in", [BL, 128, NBLK], F32, kind="ExternalInput")
    h1t_t = nc.dram_tensor("h1t_in", [128, BL], F32, kind="ExternalInput")
    vwt_t = nc.dram_tensor("vwt_in", [128, 128], dtm, kind="ExternalInput")
    wawt_t = nc.dram_tensor("wawt_in", [128, 128], F32, kind="ExternalInput")
    vb_t = nc.dram_tensor("vb_in", [128, 1], F32, kind="ExternalInput")
    wab_t = nc.dram_tensor("wab_in", [128, 1], F32, kind="ExternalInput")
    v_t = nc.dram_tensor("v_in", [128, 1], dtm, kind="ExternalInput")
    c_t = nc.dram_tensor("c_in", [1, 1], F32, kind="ExternalInput")
    eye4_t = nc.dram_tensor("eye4_in", [4, 4], F32, kind="ExternalInput")

    beta_t = nc.dram_tensor("beta_out", [BL, 128, NBLK], F32, kind="ExternalOutput")
    z_t = nc.dram_tensor("z_out", [1, BL * D], F32, kind="ExternalOutput")

    Tanh = mybir.ActivationFunctionType.Tanh
    Exp = mybir.ActivationFunctionType.Exp
    Add = mybir.AluOpType.add
    Max = mybir.AluOpType.max
    Mult = mybir.AluOpType.mult

    with PatchedTileContext(nc) as tc:
        with (
            tc.tile_pool(name="const", bufs=1) as const,
            tc.tile_pool(name="sball", bufs=1) as sball,
            tc.tile_pool(name="ht", bufs=2 if dtm == F32 else 4) as ht_pool,
            tc.tile_pool(name="g", bufs=2 if dtm != F32 else 1) as g_pool,
            tc.tile_pool(name="hn", bufs=2 if dtm == F32 else 4) as hn_pool,
            tc.tile_pool(name="ps1", bufs=2, space="PSUM") as ps1_pool,
            tc.tile_pool(name="pssm", bufs=3, space="PSUM") as pssm,
        ):
            # ---- constants / bias precompute ----
            vwt_sb = const.tile([128, 128], dtm)
            nc.sync.dma_start(out=vwt_sb[:], in_=vwt_t[:, :])
            wawt_sb = const.tile([128, 128], F32)
            nc.sync.dma_start(out=wawt_sb[:], in_=wawt_t[:, :])
            h1t_sb = const.tile([128, BL], F32)
            nc.sync.dma_start(out=h1t_sb[:], in_=h1t_t[:, :])
            vb_sb = const.tile([128, 1], F32)
            nc.sync.dma_start(out=vb_sb[:], in_=vb_t[:, :])
            wab_sb = const.tile([128, 1], F32)
            nc.sync.dma_start(out=wab_sb[:], in_=wab_t[:, :])
            v_sb = const.tile([128, 1], dtm)
            nc.sync.dma_start(out=v_sb[:], in_=v_t[:, :])
            c_bc = const.tile([128, 1], F32)
            nc.gpsimd.dma_start(out=c_bc[:], in_=c_t[:, :].to_broadcast([128, 1]))
            mask_sb = const.tile([128, BL, NBLK], F32)
            nc.sync.dma_start(
                out=mask_sb[:], in_=mask_t[:, :, :].rearrange("b p k -> p b k")
            )
            ones_sq = const.tile([128, 128], F32)
            nc.vector.memset(ones_sq[:], 1.0)
            basis4 = const.tile([4, 4], F32)
            nc.sync.dma_start(out=basis4[:], in_=eye4_t[:, :])

            # bias[o, b] = V_b[o] + Wa_b[o] + (h_tm1 @ Wa_w.T)[b, o]
            fkp_ps = pssm.tile([128, BL], F32, tag="sc", bufs=2)
            nc.tensor.matmul(fkp_ps[:], wawt_sb[:], h1t_sb[:], start=True, stop=True)
            bias_sb = const.tile([128, BL], F32)
            nc.vector.tensor_scalar(
                bias_sb[:], fkp_ps[:], vb_sb[:, 0:1], wab_sb[:, 0:1], Add, Add
            )

            def emit_body():
                _emit_main(nc, tc, dtm, sball, ht_pool, g_pool, hn_pool,
                           ps1_pool, pssm, vwt_sb, v_sb, bias_sb, c_bc,
                           mask_sb, ones_sq, basis4, ht_t, hn_t,
                           beta_t, z_t, Tanh, Exp, Add, Max, Mult)

            if reps == 1:
                emit_body()
            else:
                with tc.For_i(0, reps, 1):
                    emit_body()

    return nc


def _emit_main(nc, tc, dtm, sball, ht_pool, g_pool, hn_pool, ps1_pool, pssm,
               vwt_sb, v_sb, bias_sb, c_bc, mask_sb, ones_sq, basis4,
               ht_t, hn_t, beta_t, z_t, Tanh, Exp, Add, Max, Mult):
    Min = mybir.AluOpType.min
    z_sb = sball.tile([1, BL * D], F32)
    if ABLATE == "noz":
        nc.vector.memset(z_sb[:], 0.0)
    for b in range(BL):
        # ---- phase 1: scores for row b ----
        ht = ht_pool.tile([128, T], dtm, tag="ht")
        nc.sync.dma_start(out=ht[:], in_=ht_t[b, :, :])
        g = g_pool.tile([128, T], dtm, tag="g")
        for quad in range(4):
            c0 = quad * 2048
            ps1 = ps1_pool.tile([128, 2048], F32, tag="ps1", bufs=1)
            for h in range(4):
                nc.tensor.matmul(
                    ps1[:, h * 512 : (h + 1) * 512],
                    vwt_sb[:],
                    ht[:, c0 + h * 512 : c0 + (h + 1) * 512],
                    start=True, stop=True,
                )
            nc.scalar.activation(
                out=g[:, c0 : c0 + 2048], in_=ps1[:],
                func=Tanh, bias=bias_sb[:, b : b + 1], scale=1.0,
            )
        sc_ps = pssm.tile([128, NBLK], F32, tag="sc", bufs=2)
        for k in range(NBLK):
            nc.tensor.matmul(
                sc_ps[:, k : k + 1],
                g[:, k * 128 : (k + 1) * 128],
                v_sb[:],
                start=True, stop=True,
            )

        # ---- softmax for row b (no max-subtraction; see module docstring) ----
        x_b = sball.tile([128, NBLK], F32, tag="x", bufs=3)
        nc.vector.tensor_scalar(x_b[:], sc_ps[:], c_bc[:, 0:1], -15.0, Add, Max)
        nc.vector.tensor_scalar_min(x_b[:], x_b[:], 15.0)
        nc.vector.tensor_tensor(out=x_b[:], in0=x_b[:], in1=mask_sb[:, b, :], op=Mult)
        e_b = sball.tile([128, NBLK], F32, tag="e", bufs=3)
        nc.scalar.activation(out=e_b[:], in_=x_b[:], func=Exp)
        nc.vector.tensor_tensor(out=e_b[:], in0=e_b[:], in1=mask_sb[:, b, :], op=Mult)
        if dtm != F32:
            e_mm = sball.tile([128, NBLK], dtm, tag="emm", bufs=3)
            nc.vector.tensor_copy(e_mm[:], e_b[:])
        else:
            e_mm = e_b
        part_b = sball.tile([128, 1], F32, tag="part", bufs=3)
        nc.vector.tensor_reduce(part_b[:], e_b[:], axis=mybir.AxisListType.X, op=Add)
        sum_ps = pssm.tile([128, 1], F32, tag="z", bufs=2)
        nc.tensor.matmul(sum_ps[:], ones_sq[:], part_b[:], start=True, stop=True)
        r_bc = sball.tile([128, 1], F32, tag="rbc", bufs=3)
        nc.vector.tensor_scalar_add(r_bc[:], sum_ps[:], 1e-6)
        nc.vector.reciprocal(r_bc[:], r_bc[:])
        beta_b = sball.tile([128, NBLK], F32, tag="beta", bufs=3)
        nc.vector.tensor_scalar_mul(beta_b[:], e_b[:], r_bc[:, 0:1])
        nc.sync.dma_start(out=beta_t[b, :, :], in_=beta_b[:])

        if ABLATE == "noz":
            continue
        # ---- phase 2: z for row b ----
        hn = hn_pool.tile([128, NBLK, D], dtm, tag="hn")
        nc.sync.dma_start(out=hn[:], in_=hn_t[b, :, :, :])
        z_ps = pssm.tile([1, D], F32, tag="z", bufs=2)
        for k in range(NBLK):
            nc.tensor.matmul(
                z_ps[:],
                e_mm[:, k : k + 1],
                hn[:, k, :],
                start=(k == 0),
                stop=(k == NBLK - 1),
            )
        nc.vector.tensor_scalar_mul(
            z_sb[0:1, b * D : (b + 1) * D], z_ps[:], r_bc[0:1, 0:1]
        )
    nc.sync.dma_start(out=z_t[:, :], in_=z_sb[:])


_RUNNER = {}


def _get_runner(dtm_name, reps=1):
    key = f"{dtm_name}:{reps}"
    if key in _RUNNER:
        return _RUNNER[key]

    import jax
    from jax.sharding import Mesh, PartitionSpec
    from jax.experimental.shard_map import shard_map
    from concourse import bass2jax
    from concourse.bass2jax import _bass_exec_p, install_neuronx_cc_hook

    install_neuronx_cc_hook()

    dtm = getattr(mybir.dt, dtm_name)
    nc = build_kernel(dtm, reps=reps)

    partition_name = nc.partition_id_tensor.name if nc.partition_id_tensor else None
    in_names, out_names, out_avals, zero_outs = [], [], [], []
    for alloc in nc.m.functions[0].allocations:
        if not isinstance(alloc, mybir.MemoryLocationSet):
            continue
        name = alloc.memorylocations[0].name
        if alloc.kind == "ExternalInput":
            if name != partition_name:
                in_names.append(name)
        elif alloc.kind == "ExternalOutput":
            out_names.append(name)
            shape = tuple(alloc.tensor_shape)
            dtype = mybir.dt.np(alloc.dtype)
            out_avals.append(jax.core.ShapedArray(shape, dtype))
            zero_outs.append(np.zeros(shape, dtype))
    n_params = len(in_names)
    n_outs = len(out_avals)
    all_in_names = list(in_names) + list(out_names)
    if partition_name is not None:
        all_in_names.append(partition_name)

    def _body(*args):
        operands = list(args)
        if partition_name is not None:
            operands.append(bass2jax.partition_id_tensor())
        outs = _bass_exec_p.bind(
            *operands,
            out_avals=tuple(out_avals),
            in_names=tuple(all_in_names),
            out_names=tuple(out_names),
            lowering_input_output_aliases=(),
            sim_require_finite=True,
            sim_require_nnan=True,
            nc=nc,
        )
        return tuple(outs)

    devices = jax.devices()[:NCORES]
    mesh = Mesh(np.asarray(devices), ("core",))
    in_specs = (PartitionSpec("core"),) * (n_params + n_outs)
    out_specs = (PartitionSpec("core"),) * n_outs
    donate = tuple(range(n_params, n_params + n_outs))
    sharded = jax.jit(
        shard_map(
            _body, mesh=mesh, in_specs=in_specs, out_specs=out_specs, check_rep=False
        ),
        donate_argnums=donate,
        keep_unused=True,
    )

    sharded_nodonate = jax.jit(
        shard_map(
            _body, mesh=mesh, in_specs=in_specs, out_specs=out_specs, check_rep=False
        ),
        keep_unused=True,
    )

    runner = {
        "fn": sharded,
        "fn_nodonate": sharded_nodonate,
        "mesh": mesh,
        "nc": nc,
        "in_names": in_names,
        "all_in_names": all_in_names,
        "out_names": out_names,
        "partition_name": partition_name,
        "zero_outs": zero_outs,
        "out_avals": out_avals,
    }
    _RUNNER[key] = runner
    return runner


def benchmark_loop(inputs, reps=33, n=5):
    """Per-execution device time from the slope between a 1-iteration NEFF
    and a reps-iteration (hardware For_i) NEFF: RPC/dispatch floor cancels."""
    import jax
    from jax.sharding import NamedSharding, PartitionSpec

    kernel(**inputs)
    concat_in, concat_zeros = _LAST_CONCAT

    totals = {}
    r_lo = 9
    for r in (r_lo, reps):
        runner = _get_runner(MM_DTYPE, r)
        sh = NamedSharding(runner["mesh"], PartitionSpec("core"))
        dev_in = [jax.device_put(a, sh) for a in concat_in]
        dev_zeros = [jax.device_put(a, sh) for a in concat_zeros]
        fn = runner["fn_nodonate"]
        out = fn(*dev_in, *dev_zeros)
        jax.block_until_ready(out)
        ts = []
        for _ in range(n):
            t0 = time.perf_counter()
            out = fn(*dev_in, *dev_zeros)
            jax.block_until_ready(out)
            ts.append(time.perf_counter() - t0)
        totals[r] = ts
    per_exec = (min(totals[reps]) - min(totals[r_lo])) / (reps - r_lo)
    TIMING["exec_ns"] = per_exec * 1e9
    return totals, per_exec


def benchmark(inputs, n=10):
    """Time device-resident re-executions (no H2D in the timed loop)."""
    import jax
    from jax.sharding import NamedSharding, PartitionSpec

    runner = _get_runner(MM_DTYPE)
    # reuse kernel()'s host prep by tracing through it once
    kernel(**inputs)
    # rebuild the concat inputs exactly as kernel() does
    global _LAST_CONCAT
    concat_in, concat_zeros = _LAST_CONCAT
    sh = NamedSharding(runner["mesh"], PartitionSpec("core"))
    dev_in = [jax.device_put(a, sh) for a in concat_in]
    dev_zeros = [jax.device_put(a, sh) for a in concat_zeros]
    fn = runner["fn_nodonate"]
    out = fn(*dev_in, *dev_zeros)
    jax.block_until_ready(out)
    times = []
    for _ in range(n):
        t0 = time.perf_counter()
        out = fn(*dev_in, *dev_zeros)
        jax.block_until_ready(out)
        times.append(time.perf_counter() - t0)
    TIMING["exec_ns"] = min(times) * 1e9
    return times


def kernel(H_r, mask_r, h_tm1, V_w, V_b, Wa_w, Wa_b, v, c):
    np_mm = np.float32
    if MM_DTYPE == "bfloat16":
        import ml_dtypes
        np_mm = ml_dtypes.bfloat16

    runner = _get_runner(MM_DTYPE)

    H_r = np.asarray(H_r, dtype=np.float32)
    mask_f = np.asarray(mask_r).astype(np.float32)
    h_tm1 = np.asarray(h_tm1, dtype=np.float32)

    shared = {
        "vwt_in": np.ascontiguousarray(np.asarray(V_w, np.float32).T).astype(np_mm),
        "wawt_in": np.ascontiguousarray(np.asarray(Wa_w, np.float32).T),
        "vb_in": np.asarray(V_b, np.float32).reshape(128, 1),
        "wab_in": np.asarray(Wa_b, np.float32).reshape(128, 1),
        "v_in": np.asarray(v, np.float32).reshape(128, 1).astype(np_mm),
        "c_in": np.asarray(c, np.float32).reshape(1, 1),
        "eye4_in": np.eye(4, dtype=np.float32),
    }

    per_core = []
    for i in range(NCORES):
        sl = slice(i * BL, (i + 1) * BL)
        hs = H_r[sl]
        m = {
            "ht_in": np.ascontiguousarray(hs.transpose(0, 2, 1)).astype(np_mm),
            "hn_in": np.ascontiguousarray(
                hs.reshape(BL, NBLK, 128, D).transpose(0, 2, 1, 3)
            ).astype(np_mm),
            # mask_in[b, p, k] = mask[b, 128k + p]
            "mask_in": np.ascontiguousarray(
                mask_f[sl].reshape(BL, NBLK, 128).transpose(0, 2, 1)
            ),
            "h1t_in": np.ascontiguousarray(h_tm1[sl].T),
        }
        m.update(shared)
        per_core.append(m)

    concat_in = [
        np.concatenate([per_core[cix][name] for cix in range(NCORES)], axis=0)
        for name in runner["in_names"]
    ]
    concat_zeros = [
        np.zeros((NCORES * z.shape[0], *z.shape[1:]), z.dtype)
        for z in runner["zero_outs"]
    ]

    global _LAST_CONCAT
    _LAST_CONCAT = (concat_in, concat_zeros)

    t0 = time.perf_counter()
    out_arrs = runner["fn"](*concat_in, *concat_zeros)
    out_arrs = [np.asarray(o) for o in out_arrs]
    t1 = time.perf_counter()
    TIMING["runs"].append(t1 - t0)

    oix = {name: i for i, name in enumerate(runner["out_names"])}
    beta_all = out_arrs[oix["beta_out"]].reshape(NCORES, BL, 128, NBLK)
    z_all = out_arrs[oix["z_out"]].reshape(NCORES, 1, BL * D)

    beta = np.empty((B, T), np.float32)
    z = np.empty((B, D), np.float32)
    for i in range(NCORES):
        sl = slice(i * BL, (i + 1) * BL)
        # beta_p[b, p, k] = beta[b, 128k + p]
        beta[sl] = beta_all[i].transpose(0, 2, 1).reshape(BL, T)
        z[sl] = z_all[i].reshape(BL, D)
    return (z, beta)


# revision 20
# speedup vs baseline: 1.1635x; 1.1635x over previous
"""Bass/Trainium2 kernel for nn_BoundaryDecoderAttention.

Math (per batch row b):
  Fk    = tanh(H_r[b] @ V_w.T + V_b + h_tm1[b] @ Wa_w.T + Wa_b)   [T, D]
  s     = Fk @ v + c                                              [T]
  x     = clip(s, -15, 15) * m
  e     = exp(x) * m            (max-subtraction skipped: x <= 15 so exp is
                                 safe in fp32, and the 1e-6 epsilon term is
                                 ~1e-10 relative -- see analysis in test)
  beta  = e / (sum(e) + 1e-6)
  z     = beta @ H_r[b]                                           [D]

Sharding: data-parallel over batch, 8 rows per core, params replicated.
Device pipeline is fully "transposed": mm1 consumes host-pretransposed
H^T tiles (rhs), tanh runs on ACT with a per-partition bias, mm2 uses the
tanh tile as the *stationary* operand so scores are born spread across
128 partitions, and the z reduction streams natural-layout H blocks.
"""

import os
import sys
import time

for _p in ("/opt/trn_rl_repo", "/root/.axon_site/_ro/trn_rl_repo"):
    if os.path.isdir(_p) and _p not in sys.path:
        sys.path.insert(0, _p)

import numpy as np

import concourse.bass as bass
import concourse.bass_isa as bass_isa
from concourse import mybir
from concourse.tile import TileContext, ScopedClock

B, T, D = 64, 8192, 128
NCORES = 8
BL = B // NCORES          # 8 batch rows per core
NBLK = T // 128           # 64 t-blocks per row
NPAIR = 8                 # tanh super-chunks of 1024 cols
F32 = mybir.dt.float32

# compute dtype for the H-heavy matmuls ("float32" or "bfloat16")
MM_DTYPE = os.environ.get("BDA_MM_DTYPE", "bfloat16")
ABLATE = os.environ.get("BDA_ABLATE", "")

TIMING = {"exec_ns": None, "runs": []}


class PatchedTileContext(TileContext):
    """TileContext whose emitted instructions carry at most one sem wait.

    The walrus build in this container rejects instructions with more than
    one sync-wait command ("Too many sync wait commands"); excess waits are
    peeled onto dedicated same-engine NOPs placed just before the owner.
    """

    MAX_WAITS = 1

    def _lower_ordered_insts(self, ordered):
        for bbname in list(ordered.keys()):
            insts = ordered[bbname]
            new = []
            for inst in insts:
                si = inst.sync_info
                if si is not None and len(si.on_wait) > 1:
                    waits = list(si.on_wait)
                    for w in waits[:-1]:
                        nop = mybir.InstNoOp(
                            name=self.nc.get_next_instruction_name(),
                            sync_info=mybir.SyncInfo(on_wait=[w], on_update=[]),
                            bass_nofuse=True,
                            engine=inst.engine,
                        )
                        new.append(nop)
                    inst.sync_info = mybir.SyncInfo(
                        on_wait=[waits[-1]], on_update=list(si.on_update)
                    )
                new.append(inst)
            ordered[bbname] = new
        super()._lower_ordered_insts(ordered)

    def _drain_and_barrier(self, tick_clock, wait_clock):
        d0 = self.nc.sync.drain()
        wait_clock.add_sem_waits(d0.ins, ScopedClock({None: tick_clock.global_clock}))
        si0 = d0.ins.sync_info
        waits = list(si0.on_wait) if si0 is not None else []
        if len(waits) > self.MAX_WAITS:
            d0.ins.sync_info = mybir.SyncInfo(
                on_wait=waits[: self.MAX_WAITS], on_update=list(si0.on_update)
            )
            rest = waits[self.MAX_WAITS:]
            for i in range(0, len(rest), self.MAX_WAITS):
                d = self.nc.sync.drain()
                d.ins.sync_info = mybir.SyncInfo(
                    on_wait=rest[i : i + self.MAX_WAITS], on_update=[]
                )
        self.nc.all_engine_barrier()
        popped = self.nc._tile_sem_poison_stack.pop()
        assert popped is self._sem_poison
        self.nc.clear_and_free_semaphores(list(self.sems.allocated().values()))
        self.nc.all_engine_barrier()


def build_kernel(dtm, reps=1):
    nc = bass.Bass("TRN2")

    ht_t = nc.dram_tensor("ht_in", [BL, 128, T], dtm, kind="ExternalInput")
    hn_t = nc.dram_tensor("hn_in", [BL, 128, NBLK, D], dtm, kind="ExternalInput")
    mask_t = nc.dram_tensor("mask_in", [BL, 128, NBLK], F32, kind="ExternalInput")
    h1t_t = nc.dram_tensor("h1t_in", [128, BL], F32, kind="ExternalInput")
    vwt_t = nc.dram_tensor("vwt_in", [128, 128], dtm, kind="ExternalInput")
    wawt_t = nc.dram_tensor("wawt_in", [128, 128], F32, kind="ExternalInput")
    vb_t = nc.dram_tensor("vb_in", [128, 1], F32, kind="ExternalInput")
    wab_t = nc.dram_tensor("wab_in", [128, 1], F32, kind="ExternalInput")
    v_t = nc.dram_tensor("v_in", [128, 1], dtm, kind="ExternalInput")
    c_t = nc.dram_tensor("c_in", [1, 1], F32, kind="ExternalInput")
    eye4_t = nc.dram_tensor("eye4_in", [4, 4], F32, kind="ExternalInput")

    beta_t = nc.dram_tensor("beta_out", [BL, 128, NBLK], F32, kind="ExternalOutput")
    z_t = nc.dram_tensor("z_out", [1, BL * D], F32, kind="ExternalOutput")

    Tanh = mybir.ActivationFunctionType.Tanh
    Exp = mybir.ActivationFunctionType.Exp
    Add = mybir.AluOpType.add
    Max = mybir.AluOpType.max
    Mult = mybir.AluOpType.mult

    with PatchedTileContext(nc) as tc:
        with (
            tc.tile_pool(name="const", bufs=1) as const,
            tc.tile_pool(name="sball", bufs=1) as sball,
            tc.tile_pool(name="ht", bufs=2 if dtm == F32 else 4) as ht_pool,
            tc.tile_pool(name="g", bufs=2 if dtm != F32 else 1) as g_pool,
            tc.tile_pool(name="hn", bufs=2 if dtm == F32 else 4) as hn_pool,
            tc.tile_pool(name="ps1", bufs=2, space="PSUM") as ps1_pool,
            tc.tile_pool(name="pssm", bufs=3, space="PSUM") as pssm,
        ):
            # ---- constants / bias precompute ----
            vwt_sb = const.tile([128, 128], dtm)
            nc.sync.dma_start(out=vwt_sb[:], in_=vwt_t[:, :])
            wawt_sb = const.tile([128, 128], F32)
            nc.sync.dma_start(out=wawt_sb[:], in_=wawt_t[:, :])
            h1t_sb = const.tile([128, BL], F32)
            nc.sync.dma_start(out=h1t_sb[:], in_=h1t_t[:, :])
            vb_sb = const.tile([128, 1], F32)
            nc.sync.dma_start(out=vb_sb[:], in_=vb_t[:, :])
            wab_sb = const.tile([128, 1], F32)
            nc.sync.dma_start(out=wab_sb[:], in_=wab_t[:, :])
            v_sb = const.tile([128, 1], dtm)
            nc.sync.dma_start(out=v_sb[:], in_=v_t[:, :])
            c_bc = const.tile([128, 1], F32)
            nc.gpsimd.dma_start(out=c_bc[:], in_=c_t[:, :].to_broadcast([128, 1]))
            mask_sb = const.tile([128, BL, NBLK], F32)
            nc.sync.dma_start(
                out=mask_sb[:], in_=mask_t[:, :, :].rearrange("b p k -> p b k")
            )
            ones_sq = const.tile([128, 128], F32)
            nc.vector.memset(ones_sq[:], 1.0)
            basis4 = const.tile([4, 4], F32)
            nc.sync.dma_start(out=basis4[:], in_=eye4_t[:, :])

            # bias[o, b] = V_b[o] + Wa_b[o] + (h_tm1 @ Wa_w.T)[b, o]
            fkp_ps = pssm.tile([128, BL], F32, tag="sc", bufs=2)
            nc.tensor.matmul(fkp_ps[:], wawt_sb[:], h1t_sb[:], start=True, stop=True)
            bias_sb = const.tile([128, BL], F32)
            nc.vector.tensor_scalar(
                bias_sb[:], fkp_ps[:], vb_sb[:, 0:1], wab_sb[:, 0:1], Add, Add
            )

            def emit_body():
                _emit_main(nc, tc, dtm, sball, ht_pool, g_pool, hn_pool,
                           ps1_pool, pssm, vwt_sb, v_sb, bias_sb, c_bc,
                           mask_sb, ones_sq, basis4, ht_t, hn_t,
                           beta_t, z_t, Tanh, Exp, Add, Max, Mult)

            if reps == 1:
                emit_body()
            else:
                with tc.For_i(0, reps, 1):
                    emit_body()

    return nc


def _emit_main(nc, tc, dtm, sball, ht_pool, g_pool, hn_pool, ps1_pool, pssm,
               vwt_sb, v_sb, bias_sb, c_bc, mask_sb, ones_sq, basis4,
               ht_t, hn_t, beta_t, z_t, Tanh, Exp, Add, Max, Mult):
    Min = mybir.AluOpType.min
    z_sb = sball.tile([1, BL * D], F32)
    if ABLATE == "noz":
        nc.vector.memset(z_sb[:], 0.0)
    for b in range(BL):
        # ---- phase 1: scores for row b ----
        ht = ht_pool.tile([128, T], dtm, tag="ht")
        nc.sync.dma_start(out=ht[:], in_=ht_t[b, :, :])
        g = g_pool.tile([128, T], dtm, tag="g")
        for pair in range(NPAIR):
            c0 = pair * 1024
            ps1 = ps1_pool.tile([128, 1024], F32, tag="ps1")
            nc.tensor.matmul(
                ps1[:, 0:512], vwt_sb[:], ht[:, c0 : c0 + 512],
                start=True, stop=True,
            )
            nc.tensor.matmul(
                ps1[:, 512:1024], vwt_sb[:], ht[:, c0 + 512 : c0 + 1024],
                start=True, stop=True,
            )
            nc.scalar.activation(
                out=g[:, c0 : c0 + 1024], in_=ps1[:],
                func=Tanh, bias=bias_sb[:, b : b + 1], scale=1.0,
            )
        sc_ps = pssm.tile([128, NBLK], F32, tag="sc", bufs=2)
        for k in range(NBLK):
            nc.tensor.matmul(
                sc_ps[:, k : k + 1],
                g[:, k * 128 : (k + 1) * 128],
                v_sb[:],
                start=True, stop=True,
            )

        # ---- softmax for row b (no max-subtraction; see module docstring) ----
        x_b = sball.tile([128, NBLK], F32, tag="x", bufs=3)
        nc.vector.tensor_scalar(x_b[:], sc_ps[:], c_bc[:, 0:1], -15.0, Add, Max)
        nc.vector.tensor_scalar_min(x_b[:], x_b[:], 15.0)
        nc.vector.tensor_tensor(out=x_b[:], in0=x_b[:], in1=mask_sb[:, b, :], op=Mult)
        e_b = sball.tile([128, NBLK], F32, tag="e", bufs=3)
        nc.scalar.activation(out=e_b[:], in_=x_b[:], func=Exp)
        nc.vector.tensor_tensor(out=e_b[:], in0=e_b[:], in1=mask_sb[:, b, :], op=Mult)
        if dtm != F32:
            e_mm = sball.tile([128, NBLK], dtm, tag="emm", bufs=3)
            nc.vector.tensor_copy(e_mm[:], e_b[:])
        else:
            e_mm = e_b
        part_b = sball.tile([128, 1], F32, tag="part", bufs=3)
        nc.vector.tensor_reduce(part_b[:], e_b[:], axis=mybir.AxisListType.X, op=Add)
        sum_ps = pssm.tile([128, 1], F32, tag="z", bufs=2)
        nc.tensor.matmul(sum_ps[:], ones_sq[:], part_b[:], start=True, stop=True)
        r_bc = sball.tile([128, 1], F32, tag="rbc", bufs=3)
        nc.vector.tensor_scalar_add(r_bc[:], sum_ps[:], 1e-6)
        nc.vector.reciprocal(r_bc[:], r_bc[:])
        beta_b = sball.tile([128, NBLK], F32, tag="beta", bufs=3)
        nc.vector.tensor_scalar_mul(beta_b[:], e_b[:], r_bc[:, 0:1])
        nc.sync.dma_start(out=beta_t[b, :, :], in_=beta_b[:])

        if ABLATE == "noz":
            continue
        # ---- phase 2: z for row b ----
        hn = hn_pool.tile([128, NBLK, D], dtm, tag="hn")
        nc.sync.dma_start(out=hn[:], in_=hn_t[b, :, :, :])
        z_ps = pssm.tile([1, D], F32, tag="z", bufs=2)
        for k in range(NBLK):
            nc.tensor.matmul(
                z_ps[:],
                e_mm[:, k : k + 1],
                hn[:, k, :],
                start=(k == 0),
                stop=(k == NBLK - 1),
            )
        nc.vector.tensor_scalar_mul(
            z_sb[0:1, b * D : (b + 1) * D], z_ps[:], r_bc[0:1, 0:1]
        )
    nc.sync.dma_start(out=z_t[:, :], in_=z_sb[:])


_RUNNER = {}


def _get_runner(dtm_name, reps=1):
    key = f"{dtm_name}:{reps}"
    if key in _RUNNER:
        return _RUNNER[key]

    import jax
    from jax.sharding import Mesh, PartitionSpec
    from jax.experimental.shard_map import shard_map
    from concourse import bass2jax
    from concourse.bass2jax import _bass_exec_p, install_neuronx_cc_hook

    install_neuronx_cc_hook()

    dtm = getattr(mybir.dt, dtm_name)
    nc = build_kernel(dtm, reps=reps)

    partition_name = nc.partition_id_tensor.name if nc.partition_id_tensor else None
    in_names, out_names, out_avals, zero_outs = [], [], [], []
    for alloc in nc.m.functions[0].allocations:
        if not isinstance(alloc, mybir.MemoryLocationSet):
            continue
        name = alloc.memorylocations[0].name
        if alloc.kind == "ExternalInput":
            if name != partition_name:
                in_names.append(name)
        elif alloc.kind == "ExternalOutput":
            out_names.append(name)
            shape = tuple(alloc.tensor_shape)
            dtype = mybir.dt.np(alloc.dtype)
            out_avals.append(jax.core.ShapedArray(shape, dtype))
            zero_outs.append(np.zeros(shape, dtype))
    n_params = len(in_names)
    n_outs = len(out_avals)
    all_in_names = list(in_names) + list(out_names)
    if partition_name is not None:
        all_in_names.append(partition_name)

    def _body(*args):
        operands = list(args)
        if partition_name is not None:
            operands.append(bass2jax.partition_id_tensor())
        outs = _bass_exec_p.bind(
            *operands,
            out_avals=tuple(out_avals),
            in_names=tuple(all_in_names),
            out_names=tuple(out_names),
            lowering_input_output_aliases=(),
            sim_require_finite=True,
            sim_require_nnan=True,
            nc=nc,
        )
        return tuple(outs)

    devices = jax.devices()[:NCORES]
    mesh = Mesh(np.asarray(devices), ("core",))
    in_specs = (PartitionSpec("core"),) * (n_params + n_outs)
    out_specs = (PartitionSpec("core"),) * n_outs
    donate = tuple(range(n_params, n_params + n_outs))
    sharded = jax.jit(
        shard_map(
            _body, mesh=mesh, in_specs=in_specs, out_specs=out_specs, check_rep=False
        ),
        donate_argnums=donate,
        keep_unused=True,
    )

    sharded_nodonate = jax.jit(
        shard_map(
            _body, mesh=mesh, in_specs=in_specs, out_specs=out_specs, check_rep=False
        ),
        keep_unused=True,
    )

    runner = {
        "fn": sharded,
        "fn_nodonate": sharded_nodonate,
        "mesh": mesh,
        "nc": nc,
        "in_names": in_names,
        "all_in_names": all_in_names,
        "out_names": out_names,
        "partition_name": partition_name,
        "zero_outs": zero_outs,
        "out_avals": out_avals,
    }
    _RUNNER[key] = runner
    return runner


def benchmark_loop(inputs, reps=33, n=5):
    """Per-execution device time from the slope between a 1-iteration NEFF
    and a reps-iteration (hardware For_i) NEFF: RPC/dispatch floor cancels."""
    import jax
    from jax.sharding import NamedSharding, PartitionSpec

    kernel(**inputs)
    concat_in, concat_zeros = _LAST_CONCAT

    totals = {}
    r_lo = 9
    for r in (r_lo, reps):
        runner = _get_runner(MM_DTYPE, r)
        sh = NamedSharding(runner["mesh"], PartitionSpec("core"))
        dev_in = [jax.device_put(a, sh) for a in concat_in]
        dev_zeros = [jax.device_put(a, sh) for a in concat_zeros]
        fn = runner["fn_nodonate"]
        out = fn(*dev_in, *dev_zeros)
        jax.block_until_ready(out)
        ts = []
        for _ in range(n):
            t0 = time.perf_counter()
            out = fn(*dev_in, *dev_zeros)
            jax.block_until_ready(out)
            ts.append(time.perf_counter() - t0)
        totals[r] = ts
    per_exec = (min(totals[reps]) - min(totals[r_lo])) / (reps - r_lo)
    TIMING["exec_ns"] = per_exec * 1e9
    return totals, per_exec


def benchmark(inputs, n=10):
    """Time device-resident re-executions (no H2D in the timed loop)."""
    import jax
    from jax.sharding import NamedSharding, PartitionSpec

    runner = _get_runner(MM_DTYPE)
    # reuse kernel()'s host prep by tracing through it once
    kernel(**inputs)
    # rebuild the concat inputs exactly as kernel() does
    global _LAST_CONCAT
    concat_in, concat_zeros = _LAST_CONCAT
    sh = NamedSharding(runner["mesh"], PartitionSpec("core"))
    dev_in = [jax.device_put(a, sh) for a in concat_in]
    dev_zeros = [jax.device_put(a, sh) for a in concat_zeros]
    fn = runner["fn_nodonate"]
    out = fn(*dev_in, *dev_zeros)
    jax.block_until_ready(out)
    times = []
    for _ in range(n):
        t0 = time.perf_counter()
        out = fn(*dev_in, *dev_zeros)
        jax.block_until_ready(out)
        times.append(time.perf_counter() - t0)
    TIMING["exec_ns"] = min(times) * 1e9
    return times


def kernel(H_r, mask_r, h_tm1, V_w, V_b, Wa_w, Wa_b, v, c):
    np_mm = np.float32
    if MM_DTYPE == "bfloat16":
        import ml_dtypes
        np_mm = ml_dtypes.bfloat16

    runner = _get_runner(MM_DTYPE)

    H_r = np.asarray(H_r, dtype=np.float32)
    mask_f = np.asarray(mask_r).astype(np.float32)
    h_tm1 = np.asarray(h_tm1, dtype=np.float32)

    shared = {
        "vwt_in": np.ascontiguousarray(np.asarray(V_w, np.float32).T).astype(np_mm),
        "wawt_in": np.ascontiguousarray(np.asarray(Wa_w, np.float32).T),
        "vb_in": np.asarray(V_b, np.float32).reshape(128, 1),
        "wab_in": np.asarray(Wa_b, np.float32).reshape(128, 1),
        "v_in": np.asarray(v, np.float32).reshape(128, 1).astype(np_mm),
        "c_in": np.asarray(c, np.float32).reshape(1, 1),
        "eye4_in": np.eye(4, dtype=np.float32),
    }

    per_core = []
    for i in range(NCORES):
        sl = slice(i * BL, (i + 1) * BL)
        hs = H_r[sl]
        m = {
            "ht_in": np.ascontiguousarray(hs.transpose(0, 2, 1)).astype(np_mm),
            "hn_in": np.ascontiguousarray(
                hs.reshape(BL, NBLK, 128, D).transpose(0, 2, 1, 3)
            ).astype(np_mm),
            # mask_in[b, p, k] = mask[b, 128k + p]
            "mask_in": np.ascontiguousarray(
                mask_f[sl].reshape(BL, NBLK, 128).transpose(0, 2, 1)
            ),
            "h1t_in": np.ascontiguousarray(h_tm1[sl].T),
        }
        m.update(shared)
        per_core.append(m)

    concat_in = [
        np.concatenate([per_core[cix][name] for cix in range(NCORES)], axis=0)
        for name in runner["in_names"]
    ]
    concat_zeros = [
        np.zeros((NCORES * z.shape[0], *z.shape[1:]), z.dtype)
        for z in runner["zero_outs"]
    ]

    global _LAST_CONCAT
    _LAST_CONCAT = (concat_in, concat_zeros)

    t0 = time.perf_counter()
    out_arrs = runner["fn"](*concat_in, *concat_zeros)
    out_arrs = [np.asarray(o) for o in out_arrs]
    t1 = time.perf_counter()
    TIMING["runs"].append(t1 - t0)

    oix = {name: i for i, name in enumerate(runner["out_names"])}
    beta_all = out_arrs[oix["beta_out"]].reshape(NCORES, BL, 128, NBLK)
    z_all = out_arrs[oix["z_out"]].reshape(NCORES, 1, BL * D)

    beta = np.empty((B, T), np.float32)
    z = np.empty((B, D), np.float32)
    for i in range(NCORES):
        sl = slice(i * BL, (i + 1) * BL)
        # beta_p[b, p, k] = beta[b, 128k + p]
        beta[sl] = beta_all[i].transpose(0, 2, 1).reshape(BL, T)
        z[sl] = z_all[i].reshape(BL, D)
    return (z, beta)


# revision 21
# speedup vs baseline: 2.5942x; 2.2296x over previous
"""Bass/Trainium2 kernel for nn_BoundaryDecoderAttention.

Math (per batch row b):
  Fk    = tanh(H_r[b] @ V_w.T + V_b + h_tm1[b] @ Wa_w.T + Wa_b)   [T, D]
  s     = Fk @ v + c                                              [T]
  x     = clip(s, -15, 15) * m
  e     = exp(x) * m            (max-subtraction skipped: x <= 15 so exp is
                                 safe in fp32, and the 1e-6 epsilon term is
                                 ~1e-10 relative -- see analysis in test)
  beta  = e / (sum(e) + 1e-6)
  z     = beta @ H_r[b]                                           [D]

Sharding: data-parallel over batch, 8 rows per core, params replicated.
Device pipeline is fully "transposed": mm1 consumes host-pretransposed
H^T tiles (rhs), tanh runs on ACT with a per-partition bias, mm2 uses the
tanh tile as the *stationary* operand so scores are born spread across
128 partitions, and the z reduction streams natural-layout H blocks.
"""

import os
import sys
import time

for _p in ("/opt/trn_rl_repo", "/root/.axon_site/_ro/trn_rl_repo"):
    if os.path.isdir(_p) and _p not in sys.path:
        sys.path.insert(0, _p)

import numpy as np

import concourse.bass as bass
import concourse.bass_isa as bass_isa
from concourse import mybir
from concourse.tile import TileContext, ScopedClock

B, T, D = 64, 8192, 128
NCORES = 8
BL = B // NCORES          # 8 batch rows per core
NBLK = T // 128           # 64 t-blocks per row
NPAIR = 8                 # tanh super-chunks of 1024 cols
F32 = mybir.dt.float32

# compute dtype for the H-heavy matmuls ("float32" or "bfloat16")
MM_DTYPE = os.environ.get("BDA_MM_DTYPE", "bfloat16")
ABLATE = os.environ.get("BDA_ABLATE", "")

TIMING = {"exec_ns": None, "runs": []}


class PatchedTileContext(TileContext):
    """TileContext whose emitted instructions carry at most one sem wait.

    The walrus build in this container rejects instructions with more than
    one sync-wait command ("Too many sync wait commands"); excess waits are
    peeled onto dedicated same-engine NOPs placed just before the owner.
    """

    MAX_WAITS = 1

    def _lower_ordered_insts(self, ordered):
        for bbname in list(ordered.keys()):
            insts = ordered[bbname]
            new = []
            for inst in insts:
                si = inst.sync_info
                if si is not None and len(si.on_wait) > 1:
                    waits = list(si.on_wait)
                    for w in waits[:-1]:
                        nop = mybir.InstNoOp(
                            name=self.nc.get_next_instruction_name(),
                            sync_info=mybir.SyncInfo(on_wait=[w], on_update=[]),
                            bass_nofuse=True,
                            engine=inst.engine,
                        )
                        new.append(nop)
                    inst.sync_info = mybir.SyncInfo(
                        on_wait=[waits[-1]], on_update=list(si.on_update)
                    )
                new.append(inst)
            ordered[bbname] = new
        super()._lower_ordered_insts(ordered)

    def _drain_and_barrier(self, tick_clock, wait_clock):
        d0 = self.nc.sync.drain()
        wait_clock.add_sem_waits(d0.ins, ScopedClock({None: tick_clock.global_clock}))
        si0 = d0.ins.sync_info
        waits = list(si0.on_wait) if si0 is not None else []
        if len(waits) > self.MAX_WAITS:
            d0.ins.sync_info = mybir.SyncInfo(
                on_wait=waits[: self.MAX_WAITS], on_update=list(si0.on_update)
            )
            rest = waits[self.MAX_WAITS:]
            for i in range(0, len(rest), self.MAX_WAITS):
                d = self.nc.sync.drain()
                d.ins.sync_info = mybir.SyncInfo(
                    on_wait=rest[i : i + self.MAX_WAITS], on_update=[]
                )
        self.nc.all_engine_barrier()
        popped = self.nc._tile_sem_poison_stack.pop()
        assert popped is self._sem_poison
        self.nc.clear_and_free_semaphores(list(self.sems.allocated().values()))
        self.nc.all_engine_barrier()


def build_kernel(dtm, reps=1):
    nc = bass.Bass("TRN2")

    ht_t = nc.dram_tensor("ht_in", [BL, 128, T], dtm, kind="ExternalInput")
    hn_t = nc.dram_tensor("hn_in", [BL, 128, NBLK, D], dtm, kind="ExternalInput")
    mask_t = nc.dram_tensor("mask_in", [BL, 128, NBLK], F32, kind="ExternalInput")
    h1t_t = nc.dram_tensor("h1t_in", [128, BL], F32, kind="ExternalInput")
    vwt_t = nc.dram_tensor("vwt_in", [128, 128], dtm, kind="ExternalInput")
    wawt_t = nc.dram_tensor("wawt_in", [128, 128], F32, kind="ExternalInput")
    vb_t = nc.dram_tensor("vb_in", [128, 1], F32, kind="ExternalInput")
    wab_t = nc.dram_tensor("wab_in", [128, 1], F32, kind="ExternalInput")
    v_t = nc.dram_tensor("v_in", [128, 1], dtm, kind="ExternalInput")
    c_t = nc.dram_tensor("c_in", [1, 1], F32, kind="ExternalInput")
    eye4_t = nc.dram_tensor("eye4_in", [4, 4], F32, kind="ExternalInput")

    beta_t = nc.dram_tensor("beta_out", [BL, 128, NBLK], F32, kind="ExternalOutput")
    z_t = nc.dram_tensor("z_out", [1, BL * D], F32, kind="ExternalOutput")

    Tanh = mybir.ActivationFunctionType.Tanh
    Exp = mybir.ActivationFunctionType.Exp
    Add = mybir.AluOpType.add
    Max = mybir.AluOpType.max
    Mult = mybir.AluOpType.mult

    with PatchedTileContext(nc) as tc:
        with (
            tc.tile_pool(name="const", bufs=1) as const,
            tc.tile_pool(name="sball", bufs=1) as sball,
            tc.tile_pool(name="ht", bufs=2 if dtm == F32 else 4) as ht_pool,
            tc.tile_pool(name="g", bufs=3 if dtm != F32 else 1) as g_pool,
            tc.tile_pool(name="hn", bufs=2 if dtm == F32 else 3) as hn_pool,
            tc.tile_pool(name="ps1", bufs=2, space="PSUM") as ps1_pool,
            tc.tile_pool(name="pssm", bufs=3, space="PSUM") as pssm,
        ):
            # ---- constants / bias precompute ----
            vwt_sb = const.tile([128, 128], dtm)
            nc.sync.dma_start(out=vwt_sb[:], in_=vwt_t[:, :])
            wawt_sb = const.tile([128, 128], F32)
            nc.sync.dma_start(out=wawt_sb[:], in_=wawt_t[:, :])
            h1t_sb = const.tile([128, BL], F32)
            nc.sync.dma_start(out=h1t_sb[:], in_=h1t_t[:, :])
            vb_sb = const.tile([128, 1], F32)
            nc.sync.dma_start(out=vb_sb[:], in_=vb_t[:, :])
            wab_sb = const.tile([128, 1], F32)
            nc.sync.dma_start(out=wab_sb[:], in_=wab_t[:, :])
            v_sb = const.tile([128, 1], dtm)
            nc.sync.dma_start(out=v_sb[:], in_=v_t[:, :])
            c_bc = const.tile([128, 1], F32)
            nc.gpsimd.dma_start(out=c_bc[:], in_=c_t[:, :].to_broadcast([128, 1]))
            mask_sb = const.tile([128, BL, NBLK], F32)
            nc.sync.dma_start(
                out=mask_sb[:], in_=mask_t[:, :, :].rearrange("b p k -> p b k")
            )
            ones_sq = const.tile([128, 128], F32)
            nc.vector.memset(ones_sq[:], 1.0)
            basis4 = const.tile([4, 4], F32)
            nc.sync.dma_start(out=basis4[:], in_=eye4_t[:, :])

            # bias[o, b] = V_b[o] + Wa_b[o] + (h_tm1 @ Wa_w.T)[b, o]
            fkp_ps = pssm.tile([128, BL], F32, tag="sc", bufs=2)
            nc.tensor.matmul(fkp_ps[:], wawt_sb[:], h1t_sb[:], start=True, stop=True)
            bias_sb = const.tile([128, BL], F32)
            nc.vector.tensor_scalar(
                bias_sb[:], fkp_ps[:], vb_sb[:, 0:1], wab_sb[:, 0:1], Add, Add
            )

            def emit_body():
                _emit_main(nc, tc, dtm, sball, ht_pool, g_pool, hn_pool,
                           ps1_pool, pssm, vwt_sb, v_sb, bias_sb, c_bc,
                           mask_sb, ones_sq, basis4, ht_t, hn_t,
                           beta_t, z_t, Tanh, Exp, Add, Max, Mult)

            if reps == 1:
                emit_body()
            else:
                with tc.For_i(0, reps, 1):
                    emit_body()

    return nc


def _emit_main(nc, tc, dtm, sball, ht_pool, g_pool, hn_pool, ps1_pool, pssm,
               vwt_sb, v_sb, bias_sb, c_bc, mask_sb, ones_sq, basis4,
               ht_t, hn_t, beta_t, z_t, Tanh, Exp, Add, Max, Mult):
    Min = mybir.AluOpType.min
    z_sb = sball.tile([1, BL * D], F32)
    if ABLATE == "noz":
        nc.vector.memset(z_sb[:], 0.0)
    for b in range(BL):
        # ---- phase 1: scores for row b ----
        ht = ht_pool.tile([128, T], dtm, tag="ht")
        nc.sync.dma_start(out=ht[:], in_=ht_t[b, :, :])
        hn = hn_pool.tile([128, NBLK, D], dtm, tag="hn")
        nc.sync.dma_start(out=hn[:], in_=hn_t[b, :, :, :])
        g = g_pool.tile([128, T], dtm, tag="g")
        for pair in range(NPAIR):
            c0 = pair * 1024
            ps1 = ps1_pool.tile([128, 1024], F32, tag="ps1")
            nc.tensor.matmul(
                ps1[:, 0:512], vwt_sb[:], ht[:, c0 : c0 + 512],
                start=True, stop=True,
            )
            nc.tensor.matmul(
                ps1[:, 512:1024], vwt_sb[:], ht[:, c0 + 512 : c0 + 1024],
                start=True, stop=True,
            )
            nc.scalar.activation(
                out=g[:, c0 : c0 + 1024], in_=ps1[:],
                func=Tanh, bias=bias_sb[:, b : b + 1], scale=1.0,
            )
        sc_ps = pssm.tile([128, NBLK], F32, tag="sc", bufs=2)
        for k in range(NBLK):
            nc.tensor.matmul(
                sc_ps[:, k : k + 1],
                g[:, k * 128 : (k + 1) * 128],
                v_sb[:],
                start=True, stop=True,
            )

        # ---- softmax for row b (no max-subtraction; see module docstring) ----
        x_b = sball.tile([128, NBLK], F32, tag="x", bufs=3)
        nc.vector.tensor_scalar(x_b[:], sc_ps[:], c_bc[:, 0:1], -15.0, Add, Max)
        nc.vector.tensor_scalar_min(x_b[:], x_b[:], 15.0)
        nc.vector.tensor_tensor(out=x_b[:], in0=x_b[:], in1=mask_sb[:, b, :], op=Mult)
        e_b = sball.tile([128, NBLK], F32, tag="e", bufs=3)
        nc.scalar.activation(out=e_b[:], in_=x_b[:], func=Exp)
        nc.vector.tensor_tensor(out=e_b[:], in0=e_b[:], in1=mask_sb[:, b, :], op=Mult)
        if dtm != F32:
            e_mm = sball.tile([128, NBLK], dtm, tag="emm", bufs=3)
            nc.vector.tensor_copy(e_mm[:], e_b[:])
        else:
            e_mm = e_b
        part_b = sball.tile([128, 1], F32, tag="part", bufs=3)
        nc.vector.tensor_reduce(part_b[:], e_b[:], axis=mybir.AxisListType.X, op=Add)
        sum_ps = pssm.tile([128, 1], F32, tag="z", bufs=2)
        nc.tensor.matmul(sum_ps[:], ones_sq[:], part_b[:], start=True, stop=True)
        r_bc = sball.tile([128, 1], F32, tag="rbc", bufs=3)
        nc.vector.tensor_scalar_add(r_bc[:], sum_ps[:], 1e-6)
        nc.vector.reciprocal(r_bc[:], r_bc[:])
        if ABLATE == "noz":
            continue
        # ---- phase 2: z for row b ----
        z_ps = pssm.tile([1, D], F32, tag="z", bufs=2)
        for k in range(NBLK):
            nc.tensor.matmul(
                z_ps[:],
                e_mm[:, k : k + 1],
                hn[:, k, :],
                start=(k == 0),
                stop=(k == NBLK - 1),
            )
        nc.vector.tensor_scalar_mul(
            z_sb[0:1, b * D : (b + 1) * D], z_ps[:], r_bc[0:1, 0:1]
        )
        beta_b = sball.tile([128, NBLK], F32, tag="beta", bufs=3)
        nc.vector.tensor_scalar_mul(beta_b[:], e_b[:], r_bc[:, 0:1])
        nc.sync.dma_start(out=beta_t[b, :, :], in_=beta_b[:])
    nc.sync.dma_start(out=z_t[:, :], in_=z_sb[:])


_RUNNER = {}


def _get_runner(dtm_name, reps=1):
    key = f"{dtm_name}:{reps}"
    if key in _RUNNER:
        return _RUNNER[key]

    import jax
    from jax.sharding import Mesh, PartitionSpec
    from jax.experimental.shard_map import shard_map
    from concourse import bass2jax
    from concourse.bass2jax import _bass_exec_p, install_neuronx_cc_hook

    install_neuronx_cc_hook()

    dtm = getattr(mybir.dt, dtm_name)
    nc = build_kernel(dtm, reps=reps)

    partition_name = nc.partition_id_tensor.name if nc.partition_id_tensor else None
    in_names, out_names, out_avals, zero_outs = [], [], [], []
    for alloc in nc.m.functions[0].allocations:
        if not isinstance(alloc, mybir.MemoryLocationSet):
            continue
        name = alloc.memorylocations[0].name
        if alloc.kind == "ExternalInput":
            if name != partition_name:
                in_names.append(name)
        elif alloc.kind == "ExternalOutput":
            out_names.append(name)
            shape = tuple(alloc.tensor_shape)
            dtype = mybir.dt.np(alloc.dtype)
            out_avals.append(jax.core.ShapedArray(shape, dtype))
            zero_outs.append(np.zeros(shape, dtype))
    n_params = len(in_names)
    n_outs = len(out_avals)
    all_in_names = list(in_names) + list(out_names)
    if partition_name is not None:
        all_in_names.append(partition_name)

    def _body(*args):
        operands = list(args)
        if partition_name is not None:
            operands.append(bass2jax.partition_id_tensor())
        outs = _bass_exec_p.bind(
            *operands,
            out_avals=tuple(out_avals),
            in_names=tuple(all_in_names),
            out_names=tuple(out_names),
            lowering_input_output_aliases=(),
            sim_require_finite=True,
            sim_require_nnan=True,
            nc=nc,
        )
        return tuple(outs)

    devices = jax.devices()[:NCORES]
    mesh = Mesh(np.asarray(devices), ("core",))
    in_specs = (PartitionSpec("core"),) * (n_params + n_outs)
    out_specs = (PartitionSpec("core"),) * n_outs
    donate = tuple(range(n_params, n_params + n_outs))
    sharded = jax.jit(
        shard_map(
            _body, mesh=mesh, in_specs=in_specs, out_specs=out_specs, check_rep=False
        ),
        donate_argnums=donate,
        keep_unused=True,
    )

    sharded_nodonate = jax.jit(
        shard_map(
            _body, mesh=mesh, in_specs=in_specs, out_specs=out_specs, check_rep=False
        ),
        keep_unused=True,
    )

    runner = {
        "fn": sharded,
        "fn_nodonate": sharded_nodonate,
        "mesh": mesh,
        "nc": nc,
        "in_names": in_names,
        "all_in_names": all_in_names,
        "out_names": out_names,
        "partition_name": partition_name,
        "zero_outs": zero_outs,
        "out_avals": out_avals,
    }
    _RUNNER[key] = runner
    return runner


def benchmark_loop(inputs, reps=33, n=5):
    """Per-execution device time from the slope between a 1-iteration NEFF
    and a reps-iteration (hardware For_i) NEFF: RPC/dispatch floor cancels."""
    import jax
    from jax.sharding import NamedSharding, PartitionSpec

    kernel(**inputs)
    concat_in, concat_zeros = _LAST_CONCAT

    totals = {}
    r_lo = 9
    for r in (r_lo, reps):
        runner = _get_runner(MM_DTYPE, r)
        sh = NamedSharding(runner["mesh"], PartitionSpec("core"))
        dev_in = [jax.device_put(a, sh) for a in concat_in]
        dev_zeros = [jax.device_put(a, sh) for a in concat_zeros]
        fn = runner["fn_nodonate"]
        out = fn(*dev_in, *dev_zeros)
        jax.block_until_ready(out)
        ts = []
        for _ in range(n):
            t0 = time.perf_counter()
            out = fn(*dev_in, *dev_zeros)
            jax.block_until_ready(out)
            ts.append(time.perf_counter() - t0)
        totals[r] = ts
    per_exec = (min(totals[reps]) - min(totals[r_lo])) / (reps - r_lo)
    TIMING["exec_ns"] = per_exec * 1e9
    return totals, per_exec


def benchmark(inputs, n=10):
    """Time device-resident re-executions (no H2D in the timed loop)."""
    import jax
    from jax.sharding import NamedSharding, PartitionSpec

    runner = _get_runner(MM_DTYPE)
    # reuse kernel()'s host prep by tracing through it once
    kernel(**inputs)
    # rebuild the concat inputs exactly as kernel() does
    global _LAST_CONCAT
    concat_in, concat_zeros = _LAST_CONCAT
    sh = NamedSharding(runner["mesh"], PartitionSpec("core"))
    dev_in = [jax.device_put(a, sh) for a in concat_in]
    dev_zeros = [jax.device_put(a, sh) for a in concat_zeros]
    fn = runner["fn_nodonate"]
    out = fn(*dev_in, *dev_zeros)
    jax.block_until_ready(out)
    times = []
    for _ in range(n):
        t0 = time.perf_counter()
        out = fn(*dev_in, *dev_zeros)
        jax.block_until_ready(out)
        times.append(time.perf_counter() - t0)
    TIMING["exec_ns"] = min(times) * 1e9
    return times


def kernel(H_r, mask_r, h_tm1, V_w, V_b, Wa_w, Wa_b, v, c):
    np_mm = np.float32
    if MM_DTYPE == "bfloat16":
        import ml_dtypes
        np_mm = ml_dtypes.bfloat16

    runner = _get_runner(MM_DTYPE)

    H_r = np.asarray(H_r, dtype=np.float32)
    mask_f = np.asarray(mask_r).astype(np.float32)
    h_tm1 = np.asarray(h_tm1, dtype=np.float32)

    shared = {
        "vwt_in": np.ascontiguousarray(np.asarray(V_w, np.float32).T).astype(np_mm),
        "wawt_in": np.ascontiguousarray(np.asarray(Wa_w, np.float32).T),
        "vb_in": np.asarray(V_b, np.float32).reshape(128, 1),
        "wab_in": np.asarray(Wa_b, np.float32).reshape(128, 1),
        "v_in": np.asarray(v, np.float32).reshape(128, 1).astype(np_mm),
        "c_in": np.asarray(c, np.float32).reshape(1, 1),
        "eye4_in": np.eye(4, dtype=np.float32),
    }

    per_core = []
    for i in range(NCORES):
        sl = slice(i * BL, (i + 1) * BL)
        hs = H_r[sl]
        m = {
            "ht_in": np.ascontiguousarray(hs.transpose(0, 2, 1)).astype(np_mm),
            "hn_in": np.ascontiguousarray(
                hs.reshape(BL, NBLK, 128, D).transpose(0, 2, 1, 3)
            ).astype(np_mm),
            # mask_in[b, p, k] = mask[b, 128k + p]
            "mask_in": np.ascontiguousarray(
                mask_f[sl].reshape(BL, NBLK, 128).transpose(0, 2, 1)
            ),
            "h1t_in": np.ascontiguousarray(h_tm1[sl].T),
        }
        m.update(shared)
        per_core.append(m)

    concat_in = [
        np.concatenate([per_core[cix][name] for cix in range(NCORES)], axis=0)
        for name in runner["in_names"]
    ]
    concat_zeros = [
        np.zeros((NCORES * z.shape[0], *z.shape[1:]), z.dtype)
        for z in runner["zero_outs"]
    ]

    global _LAST_CONCAT
    _LAST_CONCAT = (concat_in, concat_zeros)

    t0 = time.perf_counter()
    out_arrs = runner["fn"](*concat_in, *concat_zeros)
    out_arrs = [np.asarray(o) for o in out_arrs]
    t1 = time.perf_counter()
    TIMING["runs"].append(t1 - t0)

    oix = {name: i for i, name in enumerate(runner["out_names"])}
    beta_all = out_arrs[oix["beta_out"]].reshape(NCORES, BL, 128, NBLK)
    z_all = out_arrs[oix["z_out"]].reshape(NCORES, 1, BL * D)

    beta = np.empty((B, T), np.float32)
    z = np.empty((B, D), np.float32)
    for i in range(NCORES):
        sl = slice(i * BL, (i + 1) * BL)
        # beta_p[b, p, k] = beta[b, 128k + p]
        beta[sl] = beta_all[i].transpose(0, 2, 1).reshape(BL, T)
        z[sl] = z_all[i].reshape(BL, D)
    return (z, beta)
